# revision 29
# baseline (speedup 1.0000x reference)
"""Trainium2 Bass kernel for nn_CPCModel_50878182588587 (vq_codebook).

Computes, for inputs encodedData [B,N,D] and protos [K,D]:
  pass1: FCM memberships of v vs protos (p=2), x = 0.5*v + 0.5*(belong@protos)
  pass2: FCM memberships of x vs protos (p=2)  -> output [B,N,K]

Sharding: data-parallel over B across 8 NeuronCores; protos replicated.

v3 design: host pre-transposes/pre-scales activations (ut = 0.5 v^T bf16,
resident in SBUF) + precomputed u2 row; normalize on ACT/DVE (not GPSIMD);
explicit 3-stage software pipeline across 512-token macro-tiles:
  A(i): dist1 + w1 + s-row + isn + bcast          (PE+DVE+ACT)
  B(i): target + x-combine + x2-row               (PE+DVE+ACT)
  C(i): dist2 + w2 + normalize + out-DMA          (PE+DVE+ACT)
emitted as A(i), B(i-1), C(i-2) so the PE always has ready matmuls from
an adjacent macro while a chain (recip/bcast/x-combine) is in flight --
keeps TensorE dense so the HAM clock stays at 2.4 GHz.
"""

import sys

import numpy as np

sys.path.insert(0, "/opt/trn_rl_repo")

import concourse.bass as bass  # noqa: E402
from concourse import bacc  # noqa: E402
import concourse.mybir as mybir  # noqa: E402
import concourse.tile as tile  # noqa: E402

B, N, D, K = 64, 1024, 256, 512
NCORES = 8
MACRO = 512  # tokens per macro-tile
f32 = mybir.dt.float32
bf16 = mybir.dt.bfloat16
FT = mybir.ActivationFunctionType
OP = mybir.AluOpType


def recip_fast(nc, out, in_):
    """reciprocal_approx_fast with any output dtype (wrapper asserts fp32)."""
    from concourse.dve_ops import RECIP_APPROX_FAST_CONSTS, RECIPROCAL_APPROX_FAST

    c = RECIP_APPROX_FAST_CONSTS
    return nc.vector._custom_dve(
        RECIPROCAL_APPROX_FAST, out=out, in0=in_, s0=c["s0"], s1=c["s1"], imm2=c["imm2"]
    )


def build_bass(T, do_compile=True):
    assert T % MACRO == 0
    nmacro = T // MACRO
    nc = bacc.Bacc(trn_type="TRN2")

    ut_d = nc.dram_tensor("ut", [D, T], bf16, kind="ExternalInput")  # 0.5*v^T
    # aug1r rows: [0]=0.25*||v||^2 per token, [1]=ones
    aug1r_d = nc.dram_tensor("aug1r", [2, T], bf16, kind="ExternalInput")
    ptm1_d = nc.dram_tensor("ptm1", [D, K], bf16, kind="ExternalInput")  # -protos.T
    ptm2_d = nc.dram_tensor("ptm2", [D, K], bf16, kind="ExternalInput")  # -2*protos.T
    pn_d = nc.dram_tensor("pn", [K, D], bf16, kind="ExternalInput")  # protos
    # aug1l rows: [0]=ones (pairs with u2 row), [1]=0.25*c^2
    aug1l_d = nc.dram_tensor("aug1l", [2, K], bf16, kind="ExternalInput")
    # aug2r rows: [0]=ones (pairs with x2 row), [1]=c^2
    aug2r_d = nc.dram_tensor("aug2r", [2, K], bf16, kind="ExternalInput")
    rowinit_d = nc.dram_tensor("rowinit", [2, MACRO], bf16, kind="ExternalInput")
    consts_d = nc.dram_tensor("consts", [128, 2], bf16, kind="ExternalInput")  # 1s, 2s
    onesrow_d = nc.dram_tensor("onesrow", [1, 128], bf16, kind="ExternalInput")
    out_d = nc.dram_tensor("out", [T, K], f32, kind="ExternalOutput")

    # input DMA slice boundaries per d-chunk (finer first slices so
    # macro 0 can start early; coarser later)
    LOAD_EDGES = [0, 256, 512, 1024, 2048, 3072, 4096, 6144, 8192]
    NWARM = 24  # PE warmup matmuls during the initial DMA wait

    with tile.TileContext(nc) as tc:
        with (
            tc.tile_pool(name="singles", bufs=1) as singles,
            tc.tile_pool(name="wt", bufs=8) as wtp,
            tc.tile_pool(name="bcs", bufs=3) as bcsp,
            tc.tile_pool(name="th", bufs=4) as thp,
            tc.tile_pool(name="xt", bufs=6) as xtp,
            tc.tile_pool(name="sqx", bufs=4) as sqxp,
            tc.tile_pool(name="isn", bufs=4) as isnp,
            tc.tile_pool(name="w2", bufs=8) as w2p,
            tc.tile_pool(name="ob", bufs=3) as obp,
            tc.tile_pool(name="small", bufs=32) as smallp,
            tc.tile_pool(name="sq1p", bufs=2, space="PSUM") as sq1_ps,
            tc.tile_pool(name="tgp", bufs=2, space="PSUM") as tg_ps,
            tc.tile_pool(name="ps2p", bufs=2, space="PSUM") as ps2_ps,
            tc.tile_pool(name="rwp", bufs=1, space="PSUM") as rows_ps,
            tc.tile_pool(name="bcq", bufs=1, space="PSUM") as bcq_ps,
        ):
            # ---- statics (small tensors first so the first matmuls of
            # macro 0 only wait on them + the first ut slice) ----
            ptm1_sb = []
            ptm2_sb = []
            for d2 in range(2):
                t1 = singles.tile([128, K], bf16, tag=f"ptm1_{d2}")
                nc.sync.dma_start(out=t1, in_=ptm1_d[d2 * 128 : (d2 + 1) * 128, :])
                ptm1_sb.append(t1)
            aug1l_sb = singles.tile([2, K], bf16, tag="aug1l")
            nc.sync.dma_start(out=aug1l_sb, in_=aug1l_d[:, :])
            aug1r_sb = singles.tile([2, T], bf16, tag="aug1r")
            nc.sync.dma_start(out=aug1r_sb, in_=aug1r_d[:, :])
            consts_sb = singles.tile([128, 2], bf16, tag="consts")
            nc.sync.dma_start(out=consts_sb, in_=consts_d[:, :])
            onesrow_sb = singles.tile([1, 128], bf16, tag="onesrow")
            nc.sync.dma_start(out=onesrow_sb, in_=onesrow_d[:, :])
            pn_sb = []
            for kc in range(4):
                t = singles.tile([128, D], bf16, tag=f"pn_{kc}")
                nc.sync.dma_start(out=t, in_=pn_d[kc * 128 : (kc + 1) * 128, :])
                pn_sb.append(t)
            for d2 in range(2):
                t2 = singles.tile([128, K], bf16, tag=f"ptm2_{d2}")
                nc.sync.dma_start(out=t2, in_=ptm2_d[d2 * 128 : (d2 + 1) * 128, :])
                ptm2_sb.append(t2)
            aug2r_sb = singles.tile([2, K], bf16, tag="aug2r")
            nc.sync.dma_start(out=aug2r_sb, in_=aug2r_d[:, :])
            aug2l_sb = []
            for e in range(2):
                t = singles.tile([2, MACRO], bf16, tag=f"aug2l_{e}")
                nc.sync.dma_start(out=t, in_=rowinit_d[:, :])
                aug2l_sb.append(t)
            # activations: whole 0.5*v^T resident, sliced DMAs.
            # d2 is the INNER loop so both halves of early slices land first.
            ut_sb = []
            for d2 in range(2):
                ut_t = singles.tile([128, T], bf16, tag=f"ut_{d2}")
                ut_sb.append(ut_t)
            for j in range(len(LOAD_EDGES) - 1):
                lo, hi = LOAD_EDGES[j], LOAD_EDGES[j + 1]
                for d2 in range(2):
                    nc.sync.dma_start(
                        out=ut_sb[d2][:, lo:hi],
                        in_=ut_d[d2 * 128 : (d2 + 1) * 128, lo:hi],
                    )
            ones_col = consts_sb[:, 0:1]
            twos_col = consts_sb[:, 1:2]
            # psum row accumulators: s-row at partition 0, x2-row at 32
            rows = rows_ps.tile([33, MACRO], f32, tag="rows")
            # PE warmup: dense matmuls on already-loaded statics while the
            # ut DMA streams in, so HAM is at 2.4 GHz for the real work.
            warm_ps = bcq_ps.tile([128, MACRO], f32, tag="bcq")
            for _ in range(NWARM):
                nc.tensor.matmul(
                    warm_ps, ptm1_sb[0][:, 0:128], ptm1_sb[1], start=True, stop=True
                )

            wts = {}  # macro -> [wt]*4   (A -> B)
            bcss = {}  # macro -> bcs     (A -> B)
            xts = {}  # macro -> [xt]*2   (B -> C)

            def stage_a(im):
                tok0 = im * MACRO
                utm = [ut_sb[d2][:, tok0 : tok0 + MACRO] for d2 in range(2)]
                wt = []
                for kc in range(4):
                    sqp = sq1_ps.tile([128, MACRO], f32, tag="sq1")
                    for d2 in range(2):
                        nc.tensor.matmul(
                            sqp,
                            ptm1_sb[d2][:, kc * 128 : (kc + 1) * 128],
                            utm[d2],
                            start=(d2 == 0),
                            stop=False,
                        )
                    nc.tensor.matmul(
                        sqp,
                        aug1l_sb[:, kc * 128 : (kc + 1) * 128],
                        aug1r_sb[:, tok0 : tok0 + MACRO],
                        start=False,
                        stop=True,
                    )
                    w = wtp.tile([128, MACRO], bf16, tag="wt")
                    recip_fast(nc, w, sqp)
                    wt.append(w)
                wts[im] = wt
                # s-row = 2*sum_k w1
                for kc in range(4):
                    nc.tensor.matmul(
                        rows[0:1, :], twos_col, wt[kc],
                        start=(kc == 0), stop=(kc == 3),
                    )
                isn = isnp.tile([1, MACRO], bf16, tag="isn")
                recip_fast(nc, isn, rows[0:1, :])
                bcq = bcq_ps.tile([128, MACRO], f32, tag="bcq")
                nc.tensor.matmul(bcq, onesrow_sb, isn, start=True, stop=True)
                bcs = bcsp.tile([128, MACRO], bf16, tag="bcs")
                nc.scalar.copy(out=bcs, in_=bcq)
                bcss[im] = bcs

            def stage_b(im):
                tok0 = im * MACRO
                ev = im % 2
                wt = wts.pop(im)
                bcs = bcss.pop(im)
                xt = []
                for d2 in range(2):
                    tg = tg_ps.tile([128, MACRO], f32, tag="tg")
                    for kc in range(4):
                        nc.tensor.matmul(
                            tg,
                            pn_sb[kc][:, d2 * 128 : (d2 + 1) * 128],
                            wt[kc],
                            start=(kc == 0),
                            stop=(kc == 3),
                        )
                    th = thp.tile([128, MACRO], bf16, tag="th")
                    nc.vector.tensor_mul(th, tg, bcs)
                    xtt = xtp.tile([128, MACRO], bf16, tag="xt")
                    nc.vector.tensor_add(xtt, th, ut_sb[d2][:, tok0 : tok0 + MACRO])
                    xt.append(xtt)
                    sq = sqxp.tile([128, MACRO], bf16, tag="sqx")
                    # one square on (otherwise idle) GPSIMD, one on ACT
                    if d2 == 0:
                        nc.gpsimd.tensor_mul(sq, xtt, xtt)
                    else:
                        nc.scalar.square(sq, xtt)
                    nc.tensor.matmul(
                        rows[32:33, :], ones_col, sq,
                        start=(d2 == 0), stop=(d2 == 1),
                    )
                xts[im] = xt
                nc.scalar.copy(out=aug2l_sb[ev][0:1, :], in_=rows[32:33, :])

            def stage_c(im):
                tok0 = im * MACRO
                ev = im % 2
                xt = xts.pop(im)
                ob4 = obp.tile([128, 4, K], f32, tag="ob")
                for s in range(4):
                    ps2 = ps2_ps.tile([128, K], f32, tag="ps2")
                    for d2 in range(2):
                        nc.tensor.matmul(
                            ps2,
                            xt[d2][:, s * 128 : (s + 1) * 128],
                            ptm2_sb[d2],
                            start=(d2 == 0),
                            stop=False,
                        )
                    nc.tensor.matmul(
                        ps2,
                        aug2l_sb[ev][:, s * 128 : (s + 1) * 128],
                        aug2r_sb,
                        start=False,
                        stop=True,
                    )
                    w2 = w2p.tile([128, K], f32, tag="w2")
                    recip_fast(nc, w2, ps2)
                    # s2 via activation-accumulate; the copy stages w2 into ob4
                    s2c = smallp.tile([128, 1], f32, tag="s2c")
                    nc.scalar.activation(
                        out=ob4[:, s, :], in_=w2, func=FT.Copy, accum_out=s2c
                    )
                    inv2 = smallp.tile([128, 1], f32, tag="inv2")
                    nc.vector.reciprocal_approx_fast(out=inv2, in_=s2c)
                    # normalize in place; split ACT/DVE for engine balance
                    if s % 2 == 0:
                        nc.scalar.mul(out=ob4[:, s, :], in_=ob4[:, s, :], mul=inv2)
                    else:
                        nc.vector.tensor_scalar(
                            out=ob4[:, s, :], in0=ob4[:, s, :],
                            scalar1=inv2, scalar2=None, op0=OP.mult,
                        )
                nc.sync.dma_start(
                    out=out_d[tok0 : tok0 + MACRO, :].rearrange(
                        "(s p) k -> p s k", p=128
                    ),
                    in_=ob4,
                )

            for it in range(nmacro + 2):
                if it < nmacro:
                    stage_a(it)
                if 1 <= it <= nmacro:
                    stage_b(it - 1)
                if it >= 2:
                    stage_c(it - 2)
    if do_compile:
        nc.compile()
    return nc


def static_inputs(protos):
    import ml_dtypes

    b = ml_dtypes.bfloat16
    protos = np.ascontiguousarray(protos, dtype=np.float32)
    pt = protos.T  # [D, K]
    c2 = (protos * protos).sum(axis=1).astype(np.float32)  # [K]
    aug1l = np.stack([np.ones(K, np.float32), 0.25 * c2])
    aug2r = np.stack([np.ones(K, np.float32), c2])
    rowinit = np.stack([np.zeros(MACRO, np.float32), np.ones(MACRO, np.float32)])
    consts = np.stack(
        [np.ones(128, np.float32), np.full(128, 2.0, np.float32)], axis=1
    )
    onesrow = np.ones((1, 128), np.float32)
    return {
        "ptm1": np.ascontiguousarray(-pt).astype(b),
        "ptm2": np.ascontiguousarray(-2.0 * pt).astype(b),
        "pn": protos.astype(b),
        "aug1l": np.ascontiguousarray(aug1l).astype(b),
        "aug2r": np.ascontiguousarray(aug2r).astype(b),
        "rowinit": np.ascontiguousarray(rowinit).astype(b),
        "consts": np.ascontiguousarray(consts).astype(b),
        "onesrow": onesrow.astype(b),
    }


_NC_CACHE = {}


def _get_nc(T):
    if T not in _NC_CACHE:
        _NC_CACHE[T] = build_bass(T)
    return _NC_CACHE[T]


def _run(encodedData, protos, trace=False):
    import ml_dtypes
    from concourse.bass_utils import run_bass_kernel_spmd

    b = ml_dtypes.bfloat16
    enc = np.ascontiguousarray(np.asarray(encodedData, dtype=np.float32))
    assert enc.shape == (B, N, D)
    T = (B // NCORES) * N
    nc = _get_nc(T)
    statics = static_inputs(np.asarray(protos, dtype=np.float32))
    bloc = B // NCORES
    in_maps = []
    for c in range(NCORES):
        v = enc[c * bloc : (c + 1) * bloc].reshape(T, D)
        ut = np.ascontiguousarray((0.5 * v).T).astype(b)  # [D, T]
        u2 = (0.25 * (v * v).sum(axis=1)).astype(np.float32)
        aug1r = np.ascontiguousarray(
            np.stack([u2, np.ones(T, np.float32)])
        ).astype(b)
        in_maps.append({"ut": ut, "aug1r": aug1r, **statics})
    res = run_bass_kernel_spmd(nc, in_maps, core_ids=list(range(NCORES)), trace=trace)
    out = np.empty((B, N, K), np.float32)
    for c in range(NCORES):
        out[c * bloc : (c + 1) * bloc] = res.results[c]["out"].astype(np.float32).reshape(bloc, N, K)
    return out, res


def kernel(**inputs):
    out, _ = _run(inputs["encodedData"], inputs["protos"])
    return out


def kernel_profiled(**inputs):
    out, res = _run(inputs["encodedData"], inputs["protos"], trace=True)
    return out, res


# revision 32
# speedup vs baseline: 1.2082x; 1.2082x over previous
"""Trainium2 Bass kernel for nn_CPCModel_50878182588587 (vq_codebook).

Computes, for inputs encodedData [B,N,D] and protos [K,D]:
  pass1: FCM memberships of v vs protos (p=2), x = 0.5*v + 0.5*(belong@protos)
  pass2: FCM memberships of x vs protos (p=2)  -> output [B,N,K]

Sharding: data-parallel over B across 8 NeuronCores; protos replicated.

Final design (920us baseline -> ~207us): host pre-transposes/pre-scales
activations (ut = 0.5 v^T bf16, resident in SBUF) + precomputed u2 row,
eliminating all PE transposes and the v^2 row pipeline; normalize on
ACT/DVE (GPSIMD tensor_scalar measured 7.5us/tile and serialized v1);
explicit 3-stage software pipeline across 512-token macro-tiles:
  A(i): dist1 + w1 + s-row + isn + bcast          (PE+DVE+ACT)
  B(i): target + x-combine + x2-row               (PE+DVE+ACT)
  C(i): dist2 + w2 + normalize + out-DMA          (PE+DVE+ACT)
emitted as A(i), B(i-1), C(i-2) so the PE always has ready matmuls from
an adjacent macro while a chain (recip/bcast/x-combine) is in flight --
keeps TensorE dense so the HAM clock stays at 2.4 GHz.
"""

import sys

import numpy as np

sys.path.insert(0, "/opt/trn_rl_repo")

import concourse.bass as bass  # noqa: E402
from concourse import bacc  # noqa: E402
import concourse.mybir as mybir  # noqa: E402
import concourse.tile as tile  # noqa: E402

B, N, D, K = 64, 1024, 256, 512
NCORES = 8
MACRO = 512  # tokens per macro-tile
f32 = mybir.dt.float32
bf16 = mybir.dt.bfloat16
FT = mybir.ActivationFunctionType
OP = mybir.AluOpType


def recip_fast(nc, out, in_):
    """reciprocal_approx_fast with any output dtype (wrapper asserts fp32)."""
    from concourse.dve_ops import RECIP_APPROX_FAST_CONSTS, RECIPROCAL_APPROX_FAST

    c = RECIP_APPROX_FAST_CONSTS
    return nc.vector._custom_dve(
        RECIPROCAL_APPROX_FAST, out=out, in0=in_, s0=c["s0"], s1=c["s1"], imm2=c["imm2"]
    )


def build_bass(T, do_compile=True):
    assert T % MACRO == 0
    nmacro = T // MACRO
    nc = bacc.Bacc(trn_type="TRN2")

    ut_d = nc.dram_tensor("ut", [D, T], bf16, kind="ExternalInput")  # 0.5*v^T
    # aug1r rows: [0]=0.25*||v||^2 per token, [1]=ones
    aug1r_d = nc.dram_tensor("aug1r", [2, T], bf16, kind="ExternalInput")
    ptm1_d = nc.dram_tensor("ptm1", [D, K], bf16, kind="ExternalInput")  # -protos.T
    ptm2_d = nc.dram_tensor("ptm2", [D, K], bf16, kind="ExternalInput")  # -2*protos.T
    pn_d = nc.dram_tensor("pn", [K, D], bf16, kind="ExternalInput")  # protos
    # aug1l rows: [0]=ones (pairs with u2 row), [1]=0.25*c^2
    aug1l_d = nc.dram_tensor("aug1l", [2, K], bf16, kind="ExternalInput")
    # aug2r rows: [0]=ones (pairs with x2 row), [1]=c^2
    aug2r_d = nc.dram_tensor("aug2r", [2, K], bf16, kind="ExternalInput")
    rowinit_d = nc.dram_tensor("rowinit", [2, MACRO], bf16, kind="ExternalInput")
    consts_d = nc.dram_tensor("consts", [128, 2], bf16, kind="ExternalInput")  # 1s, 2s
    onesrow_d = nc.dram_tensor("onesrow", [1, 128], bf16, kind="ExternalInput")
    out_d = nc.dram_tensor("out", [T, K], f32, kind="ExternalOutput")

    # input DMA slice boundaries per d-chunk (finer first slices so
    # macro 0 can start early; coarser later)
    LOAD_EDGES = [0, 256, 512, 1024, 2048, 3072, 4096, 6144, 8192]
    NWARM = 40  # PE warmup matmuls during the initial DMA wait

    with tile.TileContext(nc) as tc:
        with (
            tc.tile_pool(name="singles", bufs=1) as singles,
            tc.tile_pool(name="wt", bufs=8) as wtp,
            tc.tile_pool(name="bcs", bufs=3) as bcsp,
            tc.tile_pool(name="th", bufs=4) as thp,
            tc.tile_pool(name="xt", bufs=6) as xtp,
            tc.tile_pool(name="sqx", bufs=4) as sqxp,
            tc.tile_pool(name="isn", bufs=4) as isnp,
            tc.tile_pool(name="w2", bufs=8) as w2p,
            tc.tile_pool(name="ob", bufs=3) as obp,
            tc.tile_pool(name="small", bufs=32) as smallp,
            tc.tile_pool(name="sq1p", bufs=2, space="PSUM") as sq1_ps,
            tc.tile_pool(name="tgp", bufs=2, space="PSUM") as tg_ps,
            tc.tile_pool(name="ps2p", bufs=2, space="PSUM") as ps2_ps,
            tc.tile_pool(name="rwp", bufs=1, space="PSUM") as rows_ps,
            tc.tile_pool(name="bcq", bufs=1, space="PSUM") as bcq_ps,
        ):
            # ---- statics (small tensors first so the first matmuls of
            # macro 0 only wait on them + the first ut slice) ----
            ptm1_sb = []
            ptm2_sb = []
            for d2 in range(2):
                t1 = singles.tile([128, K], bf16, tag=f"ptm1_{d2}")
                nc.sync.dma_start(out=t1, in_=ptm1_d[d2 * 128 : (d2 + 1) * 128, :])
                ptm1_sb.append(t1)
            aug1l_sb = singles.tile([2, K], bf16, tag="aug1l")
            nc.sync.dma_start(out=aug1l_sb, in_=aug1l_d[:, :])
            aug1r_sb = singles.tile([2, T], bf16, tag="aug1r")
            nc.sync.dma_start(out=aug1r_sb, in_=aug1r_d[:, :])
            consts_sb = singles.tile([128, 2], bf16, tag="consts")
            nc.sync.dma_start(out=consts_sb, in_=consts_d[:, :])
            onesrow_sb = singles.tile([1, 128], bf16, tag="onesrow")
            nc.sync.dma_start(out=onesrow_sb, in_=onesrow_d[:, :])
            pn_sb = []
            for kc in range(4):
                t = singles.tile([128, D], bf16, tag=f"pn_{kc}")
                nc.sync.dma_start(out=t, in_=pn_d[kc * 128 : (kc + 1) * 128, :])
                pn_sb.append(t)
            for d2 in range(2):
                t2 = singles.tile([128, K], bf16, tag=f"ptm2_{d2}")
                nc.sync.dma_start(out=t2, in_=ptm2_d[d2 * 128 : (d2 + 1) * 128, :])
                ptm2_sb.append(t2)
            aug2r_sb = singles.tile([2, K], bf16, tag="aug2r")
            nc.sync.dma_start(out=aug2r_sb, in_=aug2r_d[:, :])
            aug2l_sb = []
            for e in range(2):
                t = singles.tile([2, MACRO], bf16, tag=f"aug2l_{e}")
                nc.sync.dma_start(out=t, in_=rowinit_d[:, :])
                aug2l_sb.append(t)
            # activations: whole 0.5*v^T resident, sliced DMAs.
            # d2 is the INNER loop so both halves of early slices land first.
            ut_sb = []
            for d2 in range(2):
                ut_t = singles.tile([128, T], bf16, tag=f"ut_{d2}")
                ut_sb.append(ut_t)
            for j in range(len(LOAD_EDGES) - 1):
                lo, hi = LOAD_EDGES[j], LOAD_EDGES[j + 1]
                for d2 in range(2):
                    nc.sync.dma_start(
                        out=ut_sb[d2][:, lo:hi],
                        in_=ut_d[d2 * 128 : (d2 + 1) * 128, lo:hi],
                    )
            ones_col = consts_sb[:, 0:1]
            twos_col = consts_sb[:, 1:2]
            # psum row accumulators: s-row at partition 0, x2-row at 32
            rows = rows_ps.tile([33, MACRO], f32, tag="rows")
            # PE warmup: dense matmuls on already-loaded statics while the
            # ut DMA streams in, so HAM is at 2.4 GHz for the real work.
            warm_ps = bcq_ps.tile([128, MACRO], f32, tag="bcq")
            for _ in range(NWARM):
                nc.tensor.matmul(
                    warm_ps, ptm1_sb[0][:, 0:128], ptm1_sb[1], start=True, stop=True
                )

            wts = {}  # macro -> [wt]*4   (A -> B)
            bcss = {}  # macro -> bcs     (A -> B)
            xts = {}  # macro -> [xt]*2   (B -> C)

            def stage_a(im):
                tok0 = im * MACRO
                utm = [ut_sb[d2][:, tok0 : tok0 + MACRO] for d2 in range(2)]
                wt = []
                for kc in range(4):
                    sqp = sq1_ps.tile([128, MACRO], f32, tag="sq1")
                    for d2 in range(2):
                        nc.tensor.matmul(
                            sqp,
                            ptm1_sb[d2][:, kc * 128 : (kc + 1) * 128],
                            utm[d2],
                            start=(d2 == 0),
                            stop=False,
                        )
                    nc.tensor.matmul(
                        sqp,
                        aug1l_sb[:, kc * 128 : (kc + 1) * 128],
                        aug1r_sb[:, tok0 : tok0 + MACRO],
                        start=False,
                        stop=True,
                    )
                    w = wtp.tile([128, MACRO], bf16, tag="wt")
                    recip_fast(nc, w, sqp)
                    wt.append(w)
                wts[im] = wt
                # s-row = 2*sum_k w1
                for kc in range(4):
                    nc.tensor.matmul(
                        rows[0:1, :], twos_col, wt[kc],
                        start=(kc == 0), stop=(kc == 3),
                    )
                isn = isnp.tile([1, MACRO], bf16, tag="isn")
                recip_fast(nc, isn, rows[0:1, :])
                bcq = bcq_ps.tile([128, MACRO], f32, tag="bcq")
                nc.tensor.matmul(bcq, onesrow_sb, isn, start=True, stop=True)
                bcs = bcsp.tile([128, MACRO], bf16, tag="bcs")
                nc.scalar.copy(out=bcs, in_=bcq)
                bcss[im] = bcs

            def stage_b(im):
                tok0 = im * MACRO
                ev = im % 2
                wt = wts.pop(im)
                bcs = bcss.pop(im)
                xt = []
                for d2 in range(2):
                    tg = tg_ps.tile([128, MACRO], f32, tag="tg")
                    for kc in range(4):
                        nc.tensor.matmul(
                            tg,
                            pn_sb[kc][:, d2 * 128 : (d2 + 1) * 128],
                            wt[kc],
                            start=(kc == 0),
                            stop=(kc == 3),
                        )
                    th = thp.tile([128, MACRO], bf16, tag="th")
                    nc.vector.tensor_mul(th, tg, bcs)
                    xtt = xtp.tile([128, MACRO], bf16, tag="xt")
                    nc.vector.tensor_add(xtt, th, ut_sb[d2][:, tok0 : tok0 + MACRO])
                    xt.append(xtt)
                    sq = sqxp.tile([128, MACRO], bf16, tag="sqx")
                    # one square on (otherwise idle) GPSIMD, one on ACT
                    if d2 == 0:
                        nc.gpsimd.tensor_mul(sq, xtt, xtt)
                    else:
                        nc.scalar.square(sq, xtt)
                    nc.tensor.matmul(
                        rows[32:33, :], ones_col, sq,
                        start=(d2 == 0), stop=(d2 == 1),
                    )
                xts[im] = xt
                nc.scalar.copy(out=aug2l_sb[ev][0:1, :], in_=rows[32:33, :])

            def stage_c(im):
                tok0 = im * MACRO
                ev = im % 2
                xt = xts.pop(im)
                ob4 = obp.tile([128, 4, K], f32, tag="ob")
                for s in range(4):
                    ps2 = ps2_ps.tile([128, K], f32, tag="ps2")
                    for d2 in range(2):
                        nc.tensor.matmul(
                            ps2,
                            xt[d2][:, s * 128 : (s + 1) * 128],
                            ptm2_sb[d2],
                            start=(d2 == 0),
                            stop=False,
                        )
                    nc.tensor.matmul(
                        ps2,
                        aug2l_sb[ev][:, s * 128 : (s + 1) * 128],
                        aug2r_sb,
                        start=False,
                        stop=True,
                    )
                    w2 = w2p.tile([128, K], f32, tag="w2")
                    recip_fast(nc, w2, ps2)
                    # s2 via activation-accumulate; the copy stages w2 into ob4
                    s2c = smallp.tile([128, 1], f32, tag="s2c")
                    nc.scalar.activation(
                        out=ob4[:, s, :], in_=w2, func=FT.Copy, accum_out=s2c
                    )
                    inv2 = smallp.tile([128, 1], f32, tag="inv2")
                    nc.vector.reciprocal_approx_fast(out=inv2, in_=s2c)
                    # normalize in place; split ACT/DVE for engine balance
                    if s % 2 == 0:
                        nc.scalar.mul(out=ob4[:, s, :], in_=ob4[:, s, :], mul=inv2)
                    else:
                        nc.vector.tensor_scalar(
                            out=ob4[:, s, :], in0=ob4[:, s, :],
                            scalar1=inv2, scalar2=None, op0=OP.mult,
                        )
                nc.sync.dma_start(
                    out=out_d[tok0 : tok0 + MACRO, :].rearrange(
                        "(s p) k -> p s k", p=128
                    ),
                    in_=ob4,
                )

            for it in range(nmacro + 2):
                if it < nmacro:
                    stage_a(it)
                if 1 <= it <= nmacro:
                    stage_b(it - 1)
                if it >= 2:
                    stage_c(it - 2)
    if do_compile:
        nc.compile()
    return nc


def static_inputs(protos):
    import ml_dtypes

    b = ml_dtypes.bfloat16
    protos = np.ascontiguousarray(protos, dtype=np.float32)
    pt = protos.T  # [D, K]
    c2 = (protos * protos).sum(axis=1).astype(np.float32)  # [K]
    aug1l = np.stack([np.ones(K, np.float32), 0.25 * c2])
    aug2r = np.stack([np.ones(K, np.float32), c2])
    rowinit = np.stack([np.zeros(MACRO, np.float32), np.ones(MACRO, np.float32)])
    consts = np.stack(
        [np.ones(128, np.float32), np.full(128, 2.0, np.float32)], axis=1
    )
    onesrow = np.ones((1, 128), np.float32)
    return {
        "ptm1": np.ascontiguousarray(-pt).astype(b),
        "ptm2": np.ascontiguousarray(-2.0 * pt).astype(b),
        "pn": protos.astype(b),
        "aug1l": np.ascontiguousarray(aug1l).astype(b),
        "aug2r": np.ascontiguousarray(aug2r).astype(b),
        "rowinit": np.ascontiguousarray(rowinit).astype(b),
        "consts": np.ascontiguousarray(consts).astype(b),
        "onesrow": onesrow.astype(b),
    }


_NC_CACHE = {}


def _get_nc(T):
    if T not in _NC_CACHE:
        _NC_CACHE[T] = build_bass(T)
    return _NC_CACHE[T]


def _run(encodedData, protos, trace=False):
    import ml_dtypes
    from concourse.bass_utils import run_bass_kernel_spmd

    b = ml_dtypes.bfloat16
    enc = np.ascontiguousarray(np.asarray(encodedData, dtype=np.float32))
    assert enc.shape == (B, N, D)
    T = (B // NCORES) * N
    nc = _get_nc(T)
    statics = static_inputs(np.asarray(protos, dtype=np.float32))
    bloc = B // NCORES
    in_maps = []
    for c in range(NCORES):
        v = enc[c * bloc : (c + 1) * bloc].reshape(T, D)
        ut = np.ascontiguousarray((0.5 * v).T).astype(b)  # [D, T]
        u2 = (0.25 * (v * v).sum(axis=1)).astype(np.float32)
        aug1r = np.ascontiguousarray(
            np.stack([u2, np.ones(T, np.float32)])
        ).astype(b)
        in_maps.append({"ut": ut, "aug1r": aug1r, **statics})
    res = run_bass_kernel_spmd(nc, in_maps, core_ids=list(range(NCORES)), trace=trace)
    out = np.empty((B, N, K), np.float32)
    for c in range(NCORES):
        out[c * bloc : (c + 1) * bloc] = res.results[c]["out"].astype(np.float32).reshape(bloc, N, K)
    return out, res


def kernel(**inputs):
    out, _ = _run(inputs["encodedData"], inputs["protos"])
    return out


def kernel_profiled(**inputs):
    out, res = _run(inputs["encodedData"], inputs["protos"], trace=True)
    return out, res
